# revision 28
# baseline (speedup 1.0000x reference)
# kernel.py — Show-Attend-Tell decoder on 8 Trainium2 NeuronCores.
#
# Strategy: pure data-parallel over batch (16 rows/core, zero collectives).
# The recurrent loop computes attention + LSTM per step; the big [512,10000]
# output projection is off the recurrent path and runs as batched matmuls
# (M=128 = 8 steps x 16 batch) interleaved with the loop + after it.
#
# Layout tricks:
#  - attention tensors packed as [(b,n) partitions, e free] with n padded
#    49->64 so (16 batch x 64 n) = 8 tiles of 128 partitions.
#  - e-dim (W_v/W_h rows) permuted so W_a >= 0 entries come first; |W_a| is
#    folded into W_v/W_h; e = relu-dot-W_a becomes two fused
#    tensor_scalar(max0, *sign, accum_out) passes.
#  - softmax over n and the weighted sum z use PE matmuls with block-select
#    masks (partition-dim reductions are impossible on DVE).
#  - sigmoid computed as 0.5*tanh(0.5x)+0.5 so ACT needs only one table set.

import contextlib
import math
import numpy as np
import ml_dtypes

import concourse.bass as bass
import concourse.bacc as bacc
import concourse.mybir as mybir
import concourse.tile as tile
from concourse.bass_utils import run_bass_kernel_spmd

F32 = mybir.dt.float32
BF16 = mybir.dt.bfloat16
I32 = mybir.dt.int32
AF = mybir.ActivationFunctionType
ALU = mybir.AluOpType

B, N, D, H, V, E, T = 128, 49, 512, 512, 10000, 512, 20
NCORES = 8
BL = B // NCORES          # 16 batch rows per core
NP = 64                   # n padded to 64
NT = BL * NP // 128       # 8 (b,n)-tiles of 128 partitions
KD = D // 128             # 4 k-tiles for a 512 contraction
GN = 4 * H // 512         # 4 n-tiles for gates (2048 cols)
VN = math.ceil(V / 512)   # 20 n-tiles for vocab
ROWS = T * BL             # 320 (t*16+b) rows
GTILES = math.ceil(ROWS / 128)  # 3 gather tiles

_nc_cache: dict = {}


# --------------------------------------------------------------------------
# device program
# --------------------------------------------------------------------------
def _build(n_pos: int):
    # Bacc (not Bass): its compile() splits multi-sem waits into standalone
    # event-semaphore instructions — walrus allows at most 1 wait per inst.
    nc = bacc.Bacc(None)

    din = {}
    def dp(name, shape, dt):
        din[name] = nc.declare_dram_parameter(name, list(shape), dt, isOutput=False)
        return din[name]

    dp("wx", (128, 12, 4 * H), BF16)        # gates rhs k-tiles: ctx|emb|h
    dp("bias2", (GN, 512), BF16)            # b_ih + b_hh (reordered i,f,o,g)
    dp("wv", (128, KD, D), BF16)            # (|wa|*W_v[perm]).T k-tiled
    dp("wh", (128, KD, D), BF16)            # (|wa|*W_h[perm]).T k-tiled
    dp("wout", (128, KD, V), BF16)          # W_out.T k-tiled (streamed)
    dp("bout2", (VN, 512), BF16)            # b_out tiled [20, 512]
    dp("attw", (128, D), F32)               # att_b[n(p)] * |wa[e]|
    dp("bsel", (BL, NT, 128), BF16)         # replicate-oh selector
    dp("asel", (128, NT, BL), BF16)         # z/den selector (pad rows zeroed)
    dp("bt2", (BL, 128), F32)               # b%2 == p//64 (alpha normalize)
    dp("dsel", (BL, NT), F32)               # b//2 == tau   (alpha normalize)
    dp("selg", (GN, GN, BL), BF16)          # bias-row selector (gates)
    dp("selv", (VN, VN, 128), BF16)         # bias-row selector (vocab)
    dp("ident", (128, 128), BF16)
    dp("identf", (128, 128), F32)
    dp("ones128", (128, 1), BF16)
    dp("ones16", (1, BL), BF16)
    dp("onesM", (1, 128), BF16)
    dp("imgT", (128, KD, NT * 128), BF16)   # img.T  [d, (b n)]
    dp("img_bn", (128, NT, D), BF16)        # img    [(b n), d]
    dp("mctxT", (128, KD, BL), BF16)        # mean-context.T
    dp("cap_idx", (128, GTILES, 1), I32)    # caption indices, (t,b) order
    dp("embed", (V, E), F32)                # full embedding table

    preds_d = nc.declare_dram_parameter("preds", [ROWS, V], BF16, isOutput=True)
    alph_d = nc.declare_dram_parameter("alph", [T, BL, N], F32, isOutput=True)

    with tile.TileContext(nc) as tc:
        ctx = contextlib.ExitStack()
        with ctx:
            st = ctx.enter_context(tc.tile_pool(name="state", bufs=1))
            wk = ctx.enter_context(tc.tile_pool(name="work", bufs=2))
            ps = ctx.enter_context(tc.tile_pool(name="ps", bufs=4, space="PSUM"))
            psd = ctx.enter_context(tc.tile_pool(name="psd", bufs=2, space="PSUM"))
            psg = ctx.enter_context(tc.tile_pool(name="psg", bufs=2, space="PSUM"))

            def stile(shape, dt, tg):
                return st.tile(shape, dt, tag=tg, name=tg)

            # ---- persistent SBUF state ----
            wx_sb = stile([128, 12, 4 * H], BF16, "wx")
            wv_sb = stile([128, KD, D], BF16, "wv")
            wh_sb = stile([128, KD, D], BF16, "wh")
            bias_sb = stile([GN, 512], BF16, "bias2")
            bout_sb = stile([VN, 512], BF16, "bout2")
            attw_sb = stile([128, D], F32, "attw")
            bsel_sb = stile([BL, NT, 128], BF16, "bsel")
            asel_sb = stile([128, NT, BL], BF16, "asel")
            bt2_sb = stile([BL, 128], F32, "bt2")
            dsel_sb = stile([BL, NT], F32, "dsel")
            selg_sb = stile([GN, GN, BL], BF16, "selg")
            selv_sb = stile([VN, VN, 128], BF16, "selv")
            id_sb = stile([128, 128], BF16, "ident")
            idf_sb = stile([128, 128], F32, "identf")
            ones128_sb = stile([128, 1], BF16, "ones128")
            ones16_sb = stile([1, BL], BF16, "ones16")
            onesM_sb = stile([1, 128], BF16, "onesM")
            img_bn_sb = stile([128, NT, D], BF16, "img_bn")
            mctxT_sb = stile([128, KD, BL], BF16, "mctxT")
            fv_sb = stile([128, NT, D], BF16, "fv")
            embT_sb = stile([128, KD, GTILES * 128], BF16, "embT")
            hT_sb = stile([128, KD, ROWS], BF16, "hT")   # h.T history (lhsT)
            c_sb = stile([BL, D], F32, "c")
            exph_sb = stile([128, T, NT], F32, "exph")
            ah_sb = stile([128, T, NT], F32, "ah")
            recf_sb = stile([BL, T], F32, "recf")

            for name, t_ in [
                ("wx", wx_sb), ("wv", wv_sb), ("wh", wh_sb),
                ("bias2", bias_sb), ("bout2", bout_sb), ("attw", attw_sb),
                ("bsel", bsel_sb), ("asel", asel_sb), ("bt2", bt2_sb),
                ("dsel", dsel_sb), ("selg", selg_sb), ("selv", selv_sb),
                ("ident", id_sb), ("identf", idf_sb),
                ("ones128", ones128_sb), ("ones16", ones16_sb),
                ("onesM", onesM_sb), ("img_bn", img_bn_sb), ("mctxT", mctxT_sb),
            ]:
                nc.sync.dma_start(out=t_[:], in_=din[name][:])

            # ---- prologue: embedding gather + transpose, fv precompute ----
            with tc.tile_pool(name="prol", bufs=2) as pr:
                capi = pr.tile([128, GTILES, 1], I32, tag="capi", name="capi")
                nc.sync.dma_start(out=capi[:], in_=din["cap_idx"][:])
                for g in range(GTILES):
                    ge = pr.tile([128, E], F32, tag="ge", bufs=GTILES, name="ge")
                    nc.gpsimd.indirect_dma_start(
                        out=ge[:],
                        out_offset=None,
                        in_=din["embed"][:],
                        in_offset=bass.IndirectOffsetOnAxis(ap=capi[:, g, :], axis=0),
                    )
                    # bf16 bounce: absorbs the many DMA-queue waits so the
                    # transpose matmul has a single dependency
                    geb = pr.tile([128, E], BF16, tag="geb", name="geb")
                    nc.vector.tensor_copy(out=geb[:], in_=ge[:])
                    for kt in range(KD):
                        trp = ps.tile([128, 128], BF16, tag="ps", name="trp")
                        nc.tensor.transpose(
                            out=trp[:], in_=geb[:, kt * 128:(kt + 1) * 128],
                            identity=id_sb[:])
                        nc.vector.tensor_copy(
                            out=embT_sb[:, kt, g * 128:(g + 1) * 128], in_=trp[:])

                imgT = pr.tile([128, KD, NT * 128], BF16, tag="imgT", bufs=1,
                               name="imgT")
                nc.sync.dma_start(out=imgT[:], in_=din["imgT"][:])
                for tau in range(NT):
                    fvp = ps.tile([128, D], F32, tag="ps", name="fvp")
                    for kt in range(KD):
                        nc.tensor.matmul(
                            out=fvp[:],
                            lhsT=imgT[:, kt, tau * 128:(tau + 1) * 128],
                            rhs=wv_sb[:, kt, :],
                            start=(kt == 0), stop=(kt == KD - 1))
                    nc.vector.tensor_add(
                        out=fv_sb[:, tau, :], in0=fvp[:], in1=attw_sb[:])

            nc.vector.memset(ah_sb[:, 0, :], 0.0)

            # ---- out-projection chunk (8 steps = 128 h.T columns) ----
            def out_chunk(ci):
                c0 = ci * 128
                m = min(128, ROWS - c0)
                for nt in range(VN):
                    v0 = nt * 512
                    vw = min(512, V - v0)
                    wo = wk.tile([128, KD, 512], BF16, tag="wo", name="wo")
                    nc.sync.dma_start(
                        out=wo[:, :, :vw], in_=din["wout"][:, :, v0:v0 + vw])
                    pp = psg.tile([128, 512], F32, tag="psg", name="pp")
                    for kt in range(KD):
                        nc.tensor.matmul(
                            out=pp[:m, :vw],
                            lhsT=hT_sb[:, kt, c0:c0 + m],
                            rhs=wo[:, kt, :vw],
                            start=(kt == 0), stop=False)
                    nc.tensor.matmul(
                        out=pp[:m, :vw], lhsT=selv_sb[:, nt, :m],
                        rhs=bout_sb[:, :vw], start=False, stop=True)
                    po = wk.tile([128, 512], BF16, tag="po", name="po")
                    if nt % 2 == 0:
                        nc.scalar.copy(out=po[:m, :vw], in_=pp[:m, :vw])
                    else:
                        nc.vector.tensor_copy(out=po[:m, :vw], in_=pp[:m, :vw])
                    nc.sync.dma_start(
                        out=preds_d[c0:c0 + m, v0:v0 + vw], in_=po[:m, :vw])

            # ---- recurrent loop ----
            for t in range(T):
                zT = None
                if t > 0:
                    # oh = h @ W_h2.T   [16, 512]
                    ohp = ps.tile([128, 512], F32, tag="ps", name="ohp")
                    hcol = (t - 1) * BL
                    for kt in range(KD):
                        nc.tensor.matmul(
                            out=ohp[:BL, :],
                            lhsT=hT_sb[:, kt, hcol:hcol + BL],
                            rhs=wh_sb[:, kt, :],
                            start=(kt == 0), stop=(kt == KD - 1))
                    oh_sb = wk.tile([BL, D], BF16, tag="oh", name="oh_sb")
                    nc.scalar.copy(out=oh_sb[:], in_=ohp[:BL, :])

                    ep = wk.tile([128, NT], F32, tag="ep", name="ep")
                    en = wk.tile([128, NT], F32, tag="en", name="en")
                    S = wk.tile([128, NT, D], BF16, tag="S", bufs=1, name="S")
                    for tau in range(NT):
                        orp = ps.tile([128, 512], F32, tag="ps", name="orp")
                        nc.tensor.matmul(
                            out=orp[:], lhsT=bsel_sb[:, tau, :], rhs=oh_sb[:],
                            start=True, stop=True)
                        nc.vector.tensor_add(
                            out=S[:, tau, :], in0=fv_sb[:, tau, :], in1=orp[:])
                        # e = sum_e relu(S) * sign(w_a); S overwritten in place
                        if n_pos > 0:
                            nc.vector.tensor_scalar(
                                out=S[:, tau, 0:n_pos], in0=S[:, tau, 0:n_pos],
                                scalar1=0.0, scalar2=None,
                                op0=ALU.max, op1=ALU.add,
                                accum_out=ep[:, tau:tau + 1])
                        if n_pos < D:
                            nc.vector.tensor_scalar(
                                out=S[:, tau, n_pos:D], in0=S[:, tau, n_pos:D],
                                scalar1=0.0, scalar2=None,
                                op0=ALU.max, op1=ALU.add,
                                accum_out=en[:, tau:tau + 1])
                    if 0 < n_pos < D:
                        ee = wk.tile([128, NT], F32, tag="ee", name="ee")
                        nc.vector.tensor_sub(out=ee[:], in0=ep[:], in1=en[:])
                    else:
                        ee = ep if n_pos > 0 else en
                    nc.scalar.activation(
                        out=exph_sb[:, t, :], in_=ee[:], func=AF.Exp,
                        scale=(-1.0 if n_pos == 0 else 1.0))

                    # A_sel = asel * exp  (per-partition scalar per tau)
                    a2 = wk.tile([128, NT, BL], BF16, tag="a2", name="a2")
                    for tau in range(NT):
                        nc.vector.tensor_scalar_mul(
                            out=a2[:, tau, :], in0=asel_sb[:, tau, :],
                            scalar1=exph_sb[:, t, tau:tau + 1])
                    # den + z
                    denp = psd.tile([BL, 1], F32, tag="psden", name="denp")
                    for tau in range(NT):
                        nc.tensor.matmul(
                            out=denp[:], lhsT=a2[:, tau, :], rhs=ones128_sb[:],
                            start=(tau == 0), stop=(tau == NT - 1))
                    zp = ps.tile([128, 512], F32, tag="ps", name="zp")
                    for tau in range(NT):
                        nc.tensor.matmul(
                            out=zp[:BL, :], lhsT=a2[:, tau, :],
                            rhs=img_bn_sb[:, tau, :],
                            start=(tau == 0), stop=(tau == NT - 1))
                    nc.vector.reciprocal(out=recf_sb[:, t:t + 1], in_=denp[:])
                    # normalized alphas: rec_rep[p,tau] = 1/den[2*tau + p//64]
                    ds = wk.tile([BL, NT], F32, tag="ds", name="ds")
                    nc.vector.tensor_scalar_mul(
                        out=ds[:], in0=dsel_sb[:], scalar1=recf_sb[:, t:t + 1])
                    rrp = psd.tile([128, NT], F32, tag="psden", name="rrp")
                    nc.tensor.matmul(
                        out=rrp[:], lhsT=bt2_sb[:], rhs=ds[:],
                        start=True, stop=True)
                    nc.vector.tensor_tensor(
                        out=ah_sb[:, t, :], in0=exph_sb[:, t, :], in1=rrp[:],
                        op=ALU.mult)

                    z_sb = wk.tile([BL, D], BF16, tag="z", name="z_sb")
                    nc.scalar.mul(
                        out=z_sb[:], in_=zp[:BL, :], mul=recf_sb[:, t:t + 1])
                    # z.T for the gates lhsT
                    zT = wk.tile([128, KD, BL], BF16, tag="zT", name="zT")
                    for kt in range(KD):
                        trp = ps.tile([128, 128], BF16, tag="ps", name="trz")
                        nc.tensor.transpose(
                            out=trp[:, :BL],
                            in_=z_sb[:, kt * 128:(kt + 1) * 128],
                            identity=id_sb[:BL, :BL])
                        nc.vector.tensor_copy(out=zT[:, kt, :], in_=trp[:, :BL])

                # gates: bias + [emb|ctx|h] K-tiles, col-blocks i,f,o,g
                ecol = t * BL
                th_sb = wk.tile([BL, GN, 512], F32, tag="th", bufs=1, name="th")
                for nt in range(GN):
                    gp = psg.tile([128, 512], F32, tag="psg", name="gp")
                    n0 = nt * 512
                    mms = [(selg_sb[:, nt, :], bias_sb[:])]
                    for kt in range(KD):  # emb part
                        mms.append((embT_sb[:, kt, ecol:ecol + BL],
                                    wx_sb[:, 4 + kt, n0:n0 + 512]))
                    for kt in range(KD):  # context part
                        lhs = (mctxT_sb[:, kt, :] if t == 0 else zT[:, kt, :])
                        mms.append((lhs, wx_sb[:, kt, n0:n0 + 512]))
                    if t > 0:  # h part
                        hcol = (t - 1) * BL
                        for kt in range(KD):
                            mms.append((hT_sb[:, kt, hcol:hcol + BL],
                                        wx_sb[:, 8 + kt, n0:n0 + 512]))
                    for i, (l_, r_) in enumerate(mms):
                        nc.tensor.matmul(
                            out=gp[:BL, :], lhsT=l_, rhs=r_,
                            start=(i == 0), stop=(i == len(mms) - 1))
                    # tanh(0.5x) for i,f,o ; tanh(x) for g
                    sc = 0.5 if nt < 3 else 1.0
                    nc.scalar.activation(
                        out=th_sb[:, nt, :], in_=gp[:BL, :], func=AF.Tanh,
                        scale=sc)

                # i,f,o = 0.5*tanh+0.5 (in place)
                for nt in range(3):
                    nc.vector.tensor_scalar(
                        out=th_sb[:, nt, :], in0=th_sb[:, nt, :],
                        scalar1=0.5, scalar2=0.5, op0=ALU.mult, op1=ALU.add,
                        accum_out=None)

                ig = wk.tile([BL, D], F32, tag="ig", name="ig")
                nc.vector.tensor_mul(
                    out=ig[:], in0=th_sb[:, 0, :], in1=th_sb[:, 3, :])
                if t == 0:
                    nc.vector.tensor_copy(out=c_sb[:], in_=ig[:])
                else:
                    fc = wk.tile([BL, D], F32, tag="fc", name="fc")
                    nc.vector.tensor_mul(
                        out=fc[:], in0=th_sb[:, 1, :], in1=c_sb[:])
                    nc.vector.tensor_add(out=c_sb[:], in0=fc[:], in1=ig[:])
                tc_sb = wk.tile([BL, D], F32, tag="tc", name="tc_sb")
                nc.scalar.activation(out=tc_sb[:], in_=c_sb[:], func=AF.Tanh)
                h_sb = wk.tile([BL, D], BF16, tag="h", name="h_sb")
                nc.vector.tensor_mul(
                    out=h_sb[:], in0=th_sb[:, 2, :], in1=tc_sb[:])
                hcol = t * BL
                for kt in range(KD):
                    trp = ps.tile([128, 128], BF16, tag="ps", name="trh")
                    nc.tensor.transpose(
                        out=trp[:, :BL], in_=h_sb[:, kt * 128:(kt + 1) * 128],
                        identity=id_sb[:BL, :BL])
                    nc.vector.tensor_copy(
                        out=hT_sb[:, kt, hcol:hcol + BL], in_=trp[:, :BL])

                if t == 7:
                    out_chunk(0)
                elif t == 15:
                    out_chunk(1)

            out_chunk(2)

            # ---- alphas out: [(b~ n), t, tau] -> [t, 2*tau+b~, n] ----
            for bh in range(2):
                src = ah_sb[bh * 64:bh * 64 + N, :, :]
                dst = bass.AP(
                    tensor=alph_d, offset=bh * N,
                    ap=[[1, N], [BL * N, T], [2 * N, NT]])
                nc.sync.dma_start(out=dst, in_=src)

    nc.finalize()  # runs Bacc.compile(): wait splitting, reg alloc, DCE
    return nc


# --------------------------------------------------------------------------
# host-side staging
# --------------------------------------------------------------------------
def _to_bf16(x):
    return np.asarray(x, dtype=np.float32).astype(ml_dtypes.bfloat16)


def _ktile(x):
    """[K, M] -> [128, K//128, M]"""
    k, m = x.shape
    return np.ascontiguousarray(x.reshape(k // 128, 128, m).transpose(1, 0, 2))


def _prep_shared(inp):
    wa = np.asarray(inp["W_a"], np.float32)
    perm = np.concatenate([np.where(wa >= 0)[0], np.where(wa < 0)[0]])
    n_pos = int((wa >= 0).sum())
    wap = np.abs(wa[perm])

    wv2 = np.asarray(inp["W_v"], np.float32)[perm] * wap[:, None]
    wh2 = np.asarray(inp["W_h"], np.float32)[perm] * wap[:, None]

    wih = np.asarray(inp["W_ih"], np.float32)
    whh = np.asarray(inp["W_hh"], np.float32)
    bih = np.asarray(inp["b_ih"], np.float32)
    bhh = np.asarray(inp["b_hh"], np.float32)
    colperm = np.r_[0:512, 512:1024, 1536:2048, 1024:1536]  # i,f,o,g
    wx = np.concatenate([wih[:, :D].T, wih[:, D:].T, whh.T], axis=0)[:, colperm]
    bias_row = (bih + bhh)[colperm]

    p = np.arange(128)
    att_b = np.asarray(inp["att_b"], np.float32)
    nofp = p % 64
    attn_vals = np.where(nofp < N, att_b[np.minimum(nofp, N - 1)], 0.0)
    attw = (attn_vals[:, None] * wap[None, :]).astype(np.float32)

    bsel = np.zeros((BL, NT, 128), np.float32)
    asel = np.zeros((128, NT, BL), np.float32)
    for tau in range(NT):
        bvec = 2 * tau + p // 64
        bsel[bvec, tau, p] = 1.0
        valid = nofp < N
        asel[p[valid], tau, bvec[valid]] = 1.0
    b_ar = np.arange(BL)
    bt2 = (b_ar[:, None] % 2 == p[None, :] // 64).astype(np.float32)
    dsel = (b_ar[:, None] // 2 == np.arange(NT)[None, :]).astype(np.float32)

    bout = np.zeros(VN * 512, np.float32)
    bout[:V] = np.asarray(inp["b_out"], np.float32)

    shared = {
        "wx": _ktile(wx).astype(ml_dtypes.bfloat16),
        "bias2": _to_bf16(bias_row.reshape(GN, 512)),
        "wv": _ktile(wv2.T).astype(ml_dtypes.bfloat16),
        "wh": _ktile(wh2.T).astype(ml_dtypes.bfloat16),
        "wout": _ktile(np.asarray(inp["W_out"], np.float32).T).astype(
            ml_dtypes.bfloat16),
        "bout2": _to_bf16(bout.reshape(VN, 512)),
        "attw": attw,
        "bsel": bsel.astype(ml_dtypes.bfloat16),
        "asel": asel.astype(ml_dtypes.bfloat16),
        "bt2": bt2,
        "dsel": dsel,
        "selg": np.eye(GN, dtype=np.float32)[:, :, None].repeat(BL, 2).astype(
            ml_dtypes.bfloat16),
        "selv": np.eye(VN, dtype=np.float32)[:, :, None].repeat(128, 2).astype(
            ml_dtypes.bfloat16),
        "ident": np.eye(128, dtype=ml_dtypes.bfloat16),
        "identf": np.eye(128, dtype=np.float32),
        "ones128": np.ones((128, 1), ml_dtypes.bfloat16),
        "ones16": np.ones((1, BL), ml_dtypes.bfloat16),
        "onesM": np.ones((1, 128), ml_dtypes.bfloat16),
        "embed": np.asarray(inp["embed"], np.float32),
    }
    return shared, n_pos


def _prep_core(inp, core):
    bs = slice(core * BL, (core + 1) * BL)
    img = np.asarray(inp["img_features"], np.float32)[bs]      # [16, 49, 512]
    cap = np.asarray(inp["captions"]).astype(np.int64)[bs]     # [16, 20]

    pad = np.zeros((BL, NP, D), np.float32)
    pad[:, :N, :] = img
    flat = pad.reshape(BL * NP, D)                             # rows (b*64+n)

    img_bn = np.ascontiguousarray(flat.reshape(NT, 128, D).transpose(1, 0, 2))
    imgT = _ktile(np.ascontiguousarray(flat.T))                # [128, 4, 1024]
    mctxT = _ktile(np.ascontiguousarray(img.mean(1).T))        # [128, 4, 16]

    idx = np.zeros(GTILES * 128, np.int32)
    idx[:ROWS] = cap.T.reshape(-1)                             # (t,b) order
    cap_idx = np.ascontiguousarray(
        idx.reshape(GTILES, 128).T.reshape(128, GTILES, 1))

    return {
        "imgT": imgT.astype(ml_dtypes.bfloat16),
        "img_bn": img_bn.astype(ml_dtypes.bfloat16),
        "mctxT": mctxT.astype(ml_dtypes.bfloat16),
        "cap_idx": cap_idx,
    }


def make_in_maps(inputs):
    shared, n_pos = _prep_shared(inputs)
    maps = []
    for c in range(NCORES):
        m = dict(shared)
        m.update(_prep_core(inputs, c))
        maps.append(m)
    return maps, n_pos


def kernel(**inputs):
    maps, n_pos = make_in_maps(inputs)
    if n_pos not in _nc_cache:
        _nc_cache[n_pos] = _build(n_pos)
    nc = _nc_cache[n_pos]
    res = run_bass_kernel_spmd(nc, maps, core_ids=list(range(NCORES)))
    preds = np.zeros((B, T, V), np.float32)
    alphas = np.zeros((B, N, T), np.float32)
    for c, r in enumerate(res.results):
        bs = slice(c * BL, (c + 1) * BL)
        p = np.asarray(r["preds"]).astype(np.float32)          # [320, V]
        preds[bs] = p.reshape(T, BL, V).transpose(1, 0, 2)
        a = np.asarray(r["alph"], np.float32)                  # [T, 16, N]
        alphas[bs] = a.transpose(1, 2, 0)
    return preds, alphas


# revision 32
# speedup vs baseline: 1.1073x; 1.1073x over previous
# kernel.py — Show-Attend-Tell decoder on 8 Trainium2 NeuronCores.
#
# Strategy: pure data-parallel over batch (16 rows/core, zero collectives).
# The recurrent loop computes attention + LSTM per step; the big [512,10000]
# output projection is off the recurrent path and runs as batched matmuls
# (M=128 = 8 steps x 16 batch) interleaved with the loop + after it.
#
# Layout tricks:
#  - attention tensors packed as [(b,n) partitions, e free] with n padded
#    49->64 so (16 batch x 64 n) = 8 tiles of 128 partitions.
#  - e-dim (W_v/W_h rows) permuted so W_a >= 0 entries come first; |W_a| is
#    folded into W_v/W_h; e = relu-dot-W_a becomes two fused
#    tensor_scalar(max0, *sign, accum_out) passes.
#  - softmax over n and the weighted sum z use PE matmuls with block-select
#    masks (partition-dim reductions are impossible on DVE).
#  - sigmoid computed as 0.5*tanh(0.5x)+0.5 so ACT needs only one table set.

import contextlib
import math
import numpy as np
import ml_dtypes

import concourse.bass as bass
import concourse.bacc as bacc
import concourse.mybir as mybir
import concourse.tile as tile
from concourse.bass_utils import run_bass_kernel_spmd

F32 = mybir.dt.float32
BF16 = mybir.dt.bfloat16
I32 = mybir.dt.int32
AF = mybir.ActivationFunctionType
ALU = mybir.AluOpType

B, N, D, H, V, E, T = 128, 49, 512, 512, 10000, 512, 20
NCORES = 8
BL = B // NCORES          # 16 batch rows per core
NP = 64                   # n padded to 64
NT = BL * NP // 128       # 8 (b,n)-tiles of 128 partitions
KD = D // 128             # 4 k-tiles for a 512 contraction
GN = 4 * H // 512         # 4 n-tiles for gates (2048 cols)
VN = math.ceil(V / 512)   # 20 n-tiles for vocab
ROWS = T * BL             # 320 (t*16+b) rows
GTILES = math.ceil(ROWS / 128)  # 3 gather tiles

_nc_cache: dict = {}


# --------------------------------------------------------------------------
# device program
# --------------------------------------------------------------------------
def _build(n_pos: int):
    # Bacc (not Bass): its compile() splits multi-sem waits into standalone
    # event-semaphore instructions — walrus allows at most 1 wait per inst.
    nc = bacc.Bacc(None)

    din = {}
    def dp(name, shape, dt):
        din[name] = nc.declare_dram_parameter(name, list(shape), dt, isOutput=False)
        return din[name]

    dp("wx", (128, 12, 4 * H), BF16)        # gates rhs k-tiles: ctx|emb|h
    dp("bias2", (GN, 512), BF16)            # b_ih + b_hh (reordered i,f,o,g)
    dp("wv", (128, KD, D), BF16)            # (|wa|*W_v[perm]).T k-tiled
    dp("wh", (128, KD, D), BF16)            # (|wa|*W_h[perm]).T k-tiled
    dp("wout", (128, KD, V), BF16)          # W_out.T k-tiled (streamed)
    dp("bout2", (VN, 512), BF16)            # b_out tiled [20, 512]
    dp("attw", (128, D), F32)               # att_b[n(p)] * |wa[e]|
    dp("bsel", (BL, NT, 128), BF16)         # replicate-oh selector
    dp("asel", (128, NT, BL), BF16)         # z/den selector (pad rows zeroed)
    dp("bt2", (BL, 128), F32)               # b%2 == p//64 (alpha normalize)
    dp("dsel", (BL, NT), F32)               # b//2 == tau   (alpha normalize)
    dp("selg", (GN, GN, BL), BF16)          # bias-row selector (gates)
    dp("selv", (VN, VN, 128), BF16)         # bias-row selector (vocab)
    dp("ident", (128, 128), BF16)
    dp("identf", (128, 128), F32)
    dp("ones128", (128, 1), BF16)
    dp("ones16", (1, BL), BF16)
    dp("onesM", (1, 128), BF16)
    dp("imgT", (128, KD, NT * 128), BF16)   # img.T  [d, (b n)]
    dp("img_bn", (128, NT, D), BF16)        # img    [(b n), d]
    dp("mctxT", (128, KD, BL), BF16)        # mean-context.T
    dp("cap_idx", (128, GTILES, 1), I32)    # caption indices, (t,b) order
    dp("embed", (V, E), F32)                # full embedding table

    preds_d = nc.declare_dram_parameter("preds", [ROWS, V], BF16, isOutput=True)
    alph_d = nc.declare_dram_parameter("alph", [T, BL, N], F32, isOutput=True)

    with tile.TileContext(nc) as tc:
        ctx = contextlib.ExitStack()
        with ctx:
            st = ctx.enter_context(tc.tile_pool(name="state", bufs=1))
            wk = ctx.enter_context(tc.tile_pool(name="work", bufs=2))
            ps = ctx.enter_context(tc.tile_pool(name="ps", bufs=4, space="PSUM"))
            psd = ctx.enter_context(tc.tile_pool(name="psd", bufs=2, space="PSUM"))
            psg = ctx.enter_context(tc.tile_pool(name="psg", bufs=2, space="PSUM"))

            def stile(shape, dt, tg):
                return st.tile(shape, dt, tag=tg, name=tg)

            # ---- persistent SBUF state ----
            wx_sb = stile([128, 12, 4 * H], BF16, "wx")
            wv_sb = stile([128, KD, D], BF16, "wv")
            wh_sb = stile([128, KD, D], BF16, "wh")
            bias_sb = stile([GN, 512], BF16, "bias2")
            bout_sb = stile([VN, 512], BF16, "bout2")
            attw_sb = stile([128, D], F32, "attw")
            bsel_sb = stile([BL, NT, 128], BF16, "bsel")
            asel_sb = stile([128, NT, BL], BF16, "asel")
            bt2_sb = stile([BL, 128], F32, "bt2")
            dsel_sb = stile([BL, NT], F32, "dsel")
            selg_sb = stile([GN, GN, BL], BF16, "selg")
            selv_sb = stile([VN, VN, 128], BF16, "selv")
            id_sb = stile([128, 128], BF16, "ident")
            idf_sb = stile([128, 128], F32, "identf")
            ones128_sb = stile([128, 1], BF16, "ones128")
            ones16_sb = stile([1, BL], BF16, "ones16")
            onesM_sb = stile([1, 128], BF16, "onesM")
            img_bn_sb = stile([128, NT, D], BF16, "img_bn")
            mctxT_sb = stile([128, KD, BL], BF16, "mctxT")
            fv_sb = stile([128, NT, D], BF16, "fv")
            embT_sb = stile([128, KD, GTILES * 128], BF16, "embT")
            hT_sb = stile([128, KD, ROWS], BF16, "hT")   # h.T history (lhsT)
            c_sb = stile([BL, D], F32, "c")
            exph_sb = stile([128, T, NT], F32, "exph")
            ah_sb = stile([128, T, NT], F32, "ah")
            recf_sb = stile([BL, T], F32, "recf")

            for name, t_ in [
                ("wx", wx_sb), ("wv", wv_sb), ("wh", wh_sb),
                ("bias2", bias_sb), ("bout2", bout_sb), ("attw", attw_sb),
                ("bsel", bsel_sb), ("asel", asel_sb), ("bt2", bt2_sb),
                ("dsel", dsel_sb), ("selg", selg_sb), ("selv", selv_sb),
                ("ident", id_sb), ("identf", idf_sb),
                ("ones128", ones128_sb), ("ones16", ones16_sb),
                ("onesM", onesM_sb), ("img_bn", img_bn_sb), ("mctxT", mctxT_sb),
            ]:
                nc.sync.dma_start(out=t_[:], in_=din[name][:])

            # ---- prologue: embedding gather + transpose, fv precompute ----
            with tc.tile_pool(name="prol", bufs=2) as pr:
                capi = pr.tile([128, GTILES, 1], I32, tag="capi", name="capi")
                nc.sync.dma_start(out=capi[:], in_=din["cap_idx"][:])
                for g in range(GTILES):
                    ge = pr.tile([128, E], F32, tag="ge", bufs=GTILES, name="ge")
                    nc.gpsimd.indirect_dma_start(
                        out=ge[:],
                        out_offset=None,
                        in_=din["embed"][:],
                        in_offset=bass.IndirectOffsetOnAxis(ap=capi[:, g, :], axis=0),
                    )
                    # bf16 bounce: absorbs the many DMA-queue waits so the
                    # transpose matmul has a single dependency
                    geb = pr.tile([128, E], BF16, tag="geb", name="geb")
                    nc.vector.tensor_copy(out=geb[:], in_=ge[:])
                    for kt in range(KD):
                        trp = ps.tile([128, 128], BF16, tag="ps", name="trp")
                        nc.tensor.transpose(
                            out=trp[:], in_=geb[:, kt * 128:(kt + 1) * 128],
                            identity=id_sb[:])
                        nc.vector.tensor_copy(
                            out=embT_sb[:, kt, g * 128:(g + 1) * 128], in_=trp[:])

                imgT = pr.tile([128, KD, NT * 128], BF16, tag="imgT", bufs=1,
                               name="imgT")
                nc.sync.dma_start(out=imgT[:], in_=din["imgT"][:])
                for tau in range(NT):
                    fvp = ps.tile([128, D], F32, tag="ps", name="fvp")
                    for kt in range(KD):
                        nc.tensor.matmul(
                            out=fvp[:],
                            lhsT=imgT[:, kt, tau * 128:(tau + 1) * 128],
                            rhs=wv_sb[:, kt, :],
                            start=(kt == 0), stop=(kt == KD - 1))
                    nc.vector.tensor_add(
                        out=fv_sb[:, tau, :], in0=fvp[:], in1=attw_sb[:])

            nc.vector.memset(ah_sb[:, 0, :], 0.0)

            # ---- out-projection slices, spread across steps to keep PE warm
            def out_slice(ci, nt):
                c0 = ci * 128
                m = min(128, ROWS - c0)
                v0 = nt * 512
                vw = min(512, V - v0)
                wo = wk.tile([128, KD, 512], BF16, tag="wo", name="wo")
                nc.sync.dma_start(
                    out=wo[:, :, :vw], in_=din["wout"][:, :, v0:v0 + vw])
                pp = psg.tile([128, 512], F32, tag="psg", name="pp")
                for kt in range(KD):
                    nc.tensor.matmul(
                        out=pp[:m, :vw],
                        lhsT=hT_sb[:, kt, c0:c0 + m],
                        rhs=wo[:, kt, :vw],
                        start=(kt == 0), stop=False)
                nc.tensor.matmul(
                    out=pp[:m, :vw], lhsT=selv_sb[:, nt, :m],
                    rhs=bout_sb[:, :vw], start=False, stop=True)
                po = wk.tile([128, 512], BF16, tag="po", name="po")
                if nt % 2 == 0:
                    nc.scalar.copy(out=po[:m, :vw], in_=pp[:m, :vw])
                else:
                    nc.vector.tensor_copy(out=po[:m, :vw], in_=pp[:m, :vw])
                nc.sync.dma_start(
                    out=preds_d[c0:c0 + m, v0:v0 + vw], in_=po[:m, :vw])

            from collections import deque
            out_q = deque()

            def out_drain(k):
                for _ in range(min(k, len(out_q))):
                    out_slice(*out_q.popleft())

            # ---- recurrent loop ----
            for t in range(T):
                zT = None
                if t > 0:
                    # oh = h @ W_h2.T   [16, 512]
                    ohp = ps.tile([128, 512], F32, tag="ps", name="ohp")
                    hcol = (t - 1) * BL
                    for kt in range(KD):
                        nc.tensor.matmul(
                            out=ohp[:BL, :],
                            lhsT=hT_sb[:, kt, hcol:hcol + BL],
                            rhs=wh_sb[:, kt, :],
                            start=(kt == 0), stop=(kt == KD - 1))
                    oh_sb = wk.tile([BL, D], BF16, tag="oh", name="oh_sb")
                    nc.scalar.copy(out=oh_sb[:], in_=ohp[:BL, :])

                    ep = wk.tile([128, NT], F32, tag="ep", name="ep")
                    en = wk.tile([128, NT], F32, tag="en", name="en")
                    S = wk.tile([128, NT, D], BF16, tag="S", bufs=1, name="S")
                    for tau in range(NT):
                        orp = ps.tile([128, 512], F32, tag="ps", name="orp")
                        nc.tensor.matmul(
                            out=orp[:], lhsT=bsel_sb[:, tau, :], rhs=oh_sb[:],
                            start=True, stop=True)
                        # bf16 bounce on ScalarE so the DVE add runs 2x
                        oro = wk.tile([128, 512], BF16, tag="oro", name="oro")
                        nc.scalar.copy(out=oro[:], in_=orp[:])
                        nc.vector.tensor_add(
                            out=S[:, tau, :], in0=fv_sb[:, tau, :], in1=oro[:])
                        # e = sum_e relu(S) * sign(w_a); S overwritten in place
                        if n_pos > 0:
                            nc.vector.tensor_scalar(
                                out=S[:, tau, 0:n_pos], in0=S[:, tau, 0:n_pos],
                                scalar1=0.0, scalar2=None,
                                op0=ALU.max, op1=ALU.add,
                                accum_out=ep[:, tau:tau + 1])
                        if n_pos < D:
                            nc.vector.tensor_scalar(
                                out=S[:, tau, n_pos:D], in0=S[:, tau, n_pos:D],
                                scalar1=0.0, scalar2=None,
                                op0=ALU.max, op1=ALU.add,
                                accum_out=en[:, tau:tau + 1])
                    if 0 < n_pos < D:
                        ee = wk.tile([128, NT], F32, tag="ee", name="ee")
                        nc.vector.tensor_sub(out=ee[:], in0=ep[:], in1=en[:])
                    else:
                        ee = ep if n_pos > 0 else en
                    nc.scalar.activation(
                        out=exph_sb[:, t, :], in_=ee[:], func=AF.Exp,
                        scale=(-1.0 if n_pos == 0 else 1.0))

                    # A_sel = asel * exp  (per-partition scalar per tau)
                    a2 = wk.tile([128, NT, BL], BF16, tag="a2", name="a2")
                    for tau in range(NT):
                        nc.vector.tensor_scalar_mul(
                            out=a2[:, tau, :], in0=asel_sb[:, tau, :],
                            scalar1=exph_sb[:, t, tau:tau + 1])
                    # den + z
                    denp = psd.tile([BL, 1], F32, tag="psden", name="denp")
                    for tau in range(NT):
                        nc.tensor.matmul(
                            out=denp[:], lhsT=a2[:, tau, :], rhs=ones128_sb[:],
                            start=(tau == 0), stop=(tau == NT - 1))
                    zp = ps.tile([128, 512], F32, tag="ps", name="zp")
                    for tau in range(NT):
                        nc.tensor.matmul(
                            out=zp[:BL, :], lhsT=a2[:, tau, :],
                            rhs=img_bn_sb[:, tau, :],
                            start=(tau == 0), stop=(tau == NT - 1))
                    nc.vector.reciprocal(out=recf_sb[:, t:t + 1], in_=denp[:])
                    # normalized alphas: rec_rep[p,tau] = 1/den[2*tau + p//64]
                    ds = wk.tile([BL, NT], F32, tag="ds", name="ds")
                    nc.vector.tensor_scalar_mul(
                        out=ds[:], in0=dsel_sb[:], scalar1=recf_sb[:, t:t + 1])
                    rrp = psd.tile([128, NT], F32, tag="psden", name="rrp")
                    nc.tensor.matmul(
                        out=rrp[:], lhsT=bt2_sb[:], rhs=ds[:],
                        start=True, stop=True)
                    nc.vector.tensor_tensor(
                        out=ah_sb[:, t, :], in0=exph_sb[:, t, :], in1=rrp[:],
                        op=ALU.mult)

                    z_sb = wk.tile([BL, D], BF16, tag="z", name="z_sb")
                    nc.scalar.mul(
                        out=z_sb[:], in_=zp[:BL, :], mul=recf_sb[:, t:t + 1])
                    # z.T for the gates lhsT
                    zT = wk.tile([128, KD, BL], BF16, tag="zT", name="zT")
                    for kt in range(KD):
                        trp = ps.tile([128, 128], BF16, tag="ps", name="trz")
                        nc.tensor.transpose(
                            out=trp[:, :BL],
                            in_=z_sb[:, kt * 128:(kt + 1) * 128],
                            identity=id_sb[:BL, :BL])
                        nc.vector.tensor_copy(out=zT[:, kt, :], in_=trp[:, :BL])

                # gates: bias + [emb|ctx|h] K-tiles, col-blocks i,f,o,g
                ecol = t * BL
                th_sb = wk.tile([BL, GN, 512], F32, tag="th", bufs=1, name="th")
                for nt in range(GN):
                    gp = psg.tile([128, 512], F32, tag="psg", name="gp")
                    n0 = nt * 512
                    # z-part last: everything else is ready at step entry,
                    # so PE can accumulate while attention still runs
                    mms = [(selg_sb[:, nt, :], bias_sb[:])]
                    for kt in range(KD):  # emb part
                        mms.append((embT_sb[:, kt, ecol:ecol + BL],
                                    wx_sb[:, 4 + kt, n0:n0 + 512]))
                    if t > 0:  # h part
                        hcol = (t - 1) * BL
                        for kt in range(KD):
                            mms.append((hT_sb[:, kt, hcol:hcol + BL],
                                        wx_sb[:, 8 + kt, n0:n0 + 512]))
                    for kt in range(KD):  # context part
                        lhs = (mctxT_sb[:, kt, :] if t == 0 else zT[:, kt, :])
                        mms.append((lhs, wx_sb[:, kt, n0:n0 + 512]))
                    for i, (l_, r_) in enumerate(mms):
                        nc.tensor.matmul(
                            out=gp[:BL, :], lhsT=l_, rhs=r_,
                            start=(i == 0), stop=(i == len(mms) - 1))
                    # tanh(0.5x) for i,f,o ; tanh(x) for g
                    sc = 0.5 if nt < 3 else 1.0
                    nc.scalar.activation(
                        out=th_sb[:, nt, :], in_=gp[:BL, :], func=AF.Tanh,
                        scale=sc)

                # i,f,o = 0.5*tanh+0.5 (in place)
                for nt in range(3):
                    nc.vector.tensor_scalar(
                        out=th_sb[:, nt, :], in0=th_sb[:, nt, :],
                        scalar1=0.5, scalar2=0.5, op0=ALU.mult, op1=ALU.add,
                        accum_out=None)

                ig = wk.tile([BL, D], F32, tag="ig", name="ig")
                nc.vector.tensor_mul(
                    out=ig[:], in0=th_sb[:, 0, :], in1=th_sb[:, 3, :])
                if t == 0:
                    nc.vector.tensor_copy(out=c_sb[:], in_=ig[:])
                else:
                    fc = wk.tile([BL, D], F32, tag="fc", name="fc")
                    nc.vector.tensor_mul(
                        out=fc[:], in0=th_sb[:, 1, :], in1=c_sb[:])
                    nc.vector.tensor_add(out=c_sb[:], in0=fc[:], in1=ig[:])
                tc_sb = wk.tile([BL, D], F32, tag="tc", name="tc_sb")
                nc.scalar.activation(out=tc_sb[:], in_=c_sb[:], func=AF.Tanh)
                h_sb = wk.tile([BL, D], BF16, tag="h", name="h_sb")
                nc.vector.tensor_mul(
                    out=h_sb[:], in0=th_sb[:, 2, :], in1=tc_sb[:])
                hcol = t * BL
                for kt in range(KD):
                    trp = ps.tile([128, 128], BF16, tag="ps", name="trh")
                    nc.tensor.transpose(
                        out=trp[:, :BL], in_=h_sb[:, kt * 128:(kt + 1) * 128],
                        identity=id_sb[:BL, :BL])
                    nc.vector.tensor_copy(
                        out=hT_sb[:, kt, hcol:hcol + BL], in_=trp[:, :BL])

                if t == 7:
                    out_q.extend((0, nt) for nt in range(VN))
                elif t == 15:
                    out_q.extend((1, nt) for nt in range(VN))
                if t >= 8:
                    out_drain(4 if t < 15 else 6)

            out_q.extend((2, nt) for nt in range(VN))
            out_drain(len(out_q))

            # ---- alphas out: [(b~ n), t, tau] -> [t, 2*tau+b~, n] ----
            for bh in range(2):
                src = ah_sb[bh * 64:bh * 64 + N, :, :]
                dst = bass.AP(
                    tensor=alph_d, offset=bh * N,
                    ap=[[1, N], [BL * N, T], [2 * N, NT]])
                nc.sync.dma_start(out=dst, in_=src)

    nc.finalize()  # runs Bacc.compile(): wait splitting, reg alloc, DCE
    return nc


# --------------------------------------------------------------------------
# host-side staging
# --------------------------------------------------------------------------
def _to_bf16(x):
    return np.asarray(x, dtype=np.float32).astype(ml_dtypes.bfloat16)


def _ktile(x):
    """[K, M] -> [128, K//128, M]"""
    k, m = x.shape
    return np.ascontiguousarray(x.reshape(k // 128, 128, m).transpose(1, 0, 2))


def _prep_shared(inp):
    wa = np.asarray(inp["W_a"], np.float32)
    perm = np.concatenate([np.where(wa >= 0)[0], np.where(wa < 0)[0]])
    n_pos = int((wa >= 0).sum())
    wap = np.abs(wa[perm])

    wv2 = np.asarray(inp["W_v"], np.float32)[perm] * wap[:, None]
    wh2 = np.asarray(inp["W_h"], np.float32)[perm] * wap[:, None]

    wih = np.asarray(inp["W_ih"], np.float32)
    whh = np.asarray(inp["W_hh"], np.float32)
    bih = np.asarray(inp["b_ih"], np.float32)
    bhh = np.asarray(inp["b_hh"], np.float32)
    colperm = np.r_[0:512, 512:1024, 1536:2048, 1024:1536]  # i,f,o,g
    wx = np.concatenate([wih[:, :D].T, wih[:, D:].T, whh.T], axis=0)[:, colperm]
    bias_row = (bih + bhh)[colperm]

    p = np.arange(128)
    att_b = np.asarray(inp["att_b"], np.float32)
    nofp = p % 64
    attn_vals = np.where(nofp < N, att_b[np.minimum(nofp, N - 1)], 0.0)
    attw = (attn_vals[:, None] * wap[None, :]).astype(np.float32)

    bsel = np.zeros((BL, NT, 128), np.float32)
    asel = np.zeros((128, NT, BL), np.float32)
    for tau in range(NT):
        bvec = 2 * tau + p // 64
        bsel[bvec, tau, p] = 1.0
        valid = nofp < N
        asel[p[valid], tau, bvec[valid]] = 1.0
    b_ar = np.arange(BL)
    bt2 = (b_ar[:, None] % 2 == p[None, :] // 64).astype(np.float32)
    dsel = (b_ar[:, None] // 2 == np.arange(NT)[None, :]).astype(np.float32)

    bout = np.zeros(VN * 512, np.float32)
    bout[:V] = np.asarray(inp["b_out"], np.float32)

    shared = {
        "wx": _ktile(wx).astype(ml_dtypes.bfloat16),
        "bias2": _to_bf16(bias_row.reshape(GN, 512)),
        "wv": _ktile(wv2.T).astype(ml_dtypes.bfloat16),
        "wh": _ktile(wh2.T).astype(ml_dtypes.bfloat16),
        "wout": _ktile(np.asarray(inp["W_out"], np.float32).T).astype(
            ml_dtypes.bfloat16),
        "bout2": _to_bf16(bout.reshape(VN, 512)),
        "attw": attw,
        "bsel": bsel.astype(ml_dtypes.bfloat16),
        "asel": asel.astype(ml_dtypes.bfloat16),
        "bt2": bt2,
        "dsel": dsel,
        "selg": np.eye(GN, dtype=np.float32)[:, :, None].repeat(BL, 2).astype(
            ml_dtypes.bfloat16),
        "selv": np.eye(VN, dtype=np.float32)[:, :, None].repeat(128, 2).astype(
            ml_dtypes.bfloat16),
        "ident": np.eye(128, dtype=ml_dtypes.bfloat16),
        "identf": np.eye(128, dtype=np.float32),
        "ones128": np.ones((128, 1), ml_dtypes.bfloat16),
        "ones16": np.ones((1, BL), ml_dtypes.bfloat16),
        "onesM": np.ones((1, 128), ml_dtypes.bfloat16),
        "embed": np.asarray(inp["embed"], np.float32),
    }
    return shared, n_pos


def _prep_core(inp, core):
    bs = slice(core * BL, (core + 1) * BL)
    img = np.asarray(inp["img_features"], np.float32)[bs]      # [16, 49, 512]
    cap = np.asarray(inp["captions"]).astype(np.int64)[bs]     # [16, 20]

    pad = np.zeros((BL, NP, D), np.float32)
    pad[:, :N, :] = img
    flat = pad.reshape(BL * NP, D)                             # rows (b*64+n)

    img_bn = np.ascontiguousarray(flat.reshape(NT, 128, D).transpose(1, 0, 2))
    imgT = _ktile(np.ascontiguousarray(flat.T))                # [128, 4, 1024]
    mctxT = _ktile(np.ascontiguousarray(img.mean(1).T))        # [128, 4, 16]

    idx = np.zeros(GTILES * 128, np.int32)
    idx[:ROWS] = cap.T.reshape(-1)                             # (t,b) order
    cap_idx = np.ascontiguousarray(
        idx.reshape(GTILES, 128).T.reshape(128, GTILES, 1))

    return {
        "imgT": imgT.astype(ml_dtypes.bfloat16),
        "img_bn": img_bn.astype(ml_dtypes.bfloat16),
        "mctxT": mctxT.astype(ml_dtypes.bfloat16),
        "cap_idx": cap_idx,
    }


def make_in_maps(inputs):
    shared, n_pos = _prep_shared(inputs)
    maps = []
    for c in range(NCORES):
        m = dict(shared)
        m.update(_prep_core(inputs, c))
        maps.append(m)
    return maps, n_pos


def kernel(**inputs):
    maps, n_pos = make_in_maps(inputs)
    if n_pos not in _nc_cache:
        _nc_cache[n_pos] = _build(n_pos)
    nc = _nc_cache[n_pos]
    res = run_bass_kernel_spmd(nc, maps, core_ids=list(range(NCORES)))
    preds = np.zeros((B, T, V), np.float32)
    alphas = np.zeros((B, N, T), np.float32)
    for c, r in enumerate(res.results):
        bs = slice(c * BL, (c + 1) * BL)
        p = np.asarray(r["preds"]).astype(np.float32)          # [320, V]
        preds[bs] = p.reshape(T, BL, V).transpose(1, 0, 2)
        a = np.asarray(r["alph"], np.float32)                  # [T, 16, N]
        alphas[bs] = a.transpose(1, 2, 0)
    return preds, alphas
